# revision 13
# baseline (speedup 1.0000x reference)
"""DE/NN population-update kernel for Trainium2 (8 NeuronCores).

Reference computation (per parameter tensor p with uniform tensor ri):
    mutant = p + F*(p[best] - p) + F*(p[a0] - p[a1])        (gathers along NP axis)
    cond   = (ri < CR) | (Rs == layer)[:, None...]
    y      = where(cond, mutant, p)
    out    = where((fy <= fx)[:, None...], y, p)

Key transforms used here:
  * mutant = M @ p along the NP=44 axis with
        M[i,j] = (1-F)*d_ij + F*d[j==best] + F*d[j==a0[i]] - F*d[j==a1[i]]
    so the row gathers become one small 44x44 (stacked to 88x88 block-diag)
    matmul on the otherwise idle TensorEngine -- no extra HBM traffic.
  * cond & accept fold into a per-row threshold on ri:
        thr_i = -1.0          if fy_i > fx_i        (reject row entirely)
              =  0.9          if accept, Rs_i != L  (plain CR test)
              =  1.9          if accept, Rs_i == L  (forced crossover; ri<1)
    so per element the device does:  mask = (ri < thr_row); out = mask ? mutant : p
  * Sharding: every [NP, ...] tensor is split along flattened trailing dims
    across the 8 cores; fx/fy/min/argmin are 44-element host-side ops.
"""

import numpy as np

import concourse.bass as bass
import concourse.bacc as bacc
import concourse.mybir as mybir
from concourse.bass_utils import run_bass_kernel_spmd
from concourse.tile import TileContext

F_CONST = 0.8
CR_CONST = 0.9
NP_POP = 44
IN_D, HID, OUT_D = 512, 1024, 512
N_CORES = 8

CW0 = HID * IN_D // N_CORES     # 65536 cols/core
CW1 = HID * HID // N_CORES      # 131072
CW2 = OUT_D * HID // N_CORES    # 65536
CB0 = HID // N_CORES            # 128
CB2 = OUT_D // N_CORES          # 64
CB = 2 * CB0 + CB2              # 320 (b0|b1|b2 concat)

W = 2048                        # free-dim columns per block
MMN = 512                       # matmul moving-operand max for fp32 (1 PSUM bank)

# (name, cols-per-core, layer index) for the big weight tensors
W_SPECS = (("w0", CW0, 0), ("w1", CW1, 1), ("w2", CW2, 2))

_PROGRAM = None


def _build_program():
    """Trace the per-core Bass/Tile program (shapes only; all values are inputs)."""
    global _PROGRAM
    if _PROGRAM is not None:
        return _PROGRAM

    nc = bacc.Bacc()
    f32 = mybir.dt.float32
    P2 = 2 * NP_POP  # 88

    xw, rw, ow = {}, {}, {}
    for name, C, _ in W_SPECS:
        xw[name] = nc.dram_tensor(f"x_{name}", [NP_POP, C], f32, kind="ExternalInput")
        rw[name] = nc.dram_tensor(f"r_{name}", [NP_POP, C], f32, kind="ExternalInput")
        ow[name] = nc.dram_tensor(f"o_{name}", [NP_POP, C], f32, kind="ExternalOutput")
    xb = nc.dram_tensor("x_b", [NP_POP, CB], f32, kind="ExternalInput")
    rb = nc.dram_tensor("r_b", [NP_POP, CB], f32, kind="ExternalInput")
    ob = nc.dram_tensor("o_b", [NP_POP, CB], f32, kind="ExternalOutput")
    mt = nc.dram_tensor("mt", [P2, P2], f32, kind="ExternalInput")   # block-diag(M^T, M^T)
    th = nc.dram_tensor("th", [P2, 4], f32, kind="ExternalInput")    # col L = [thr_L; thr_L]

    with TileContext(nc) as tc:
        with (
            tc.tile_pool(name="const", bufs=1) as cpool,
            tc.tile_pool(name="xp", bufs=8) as xpool,
            tc.tile_pool(name="rp", bufs=8) as rpool,
            tc.tile_pool(name="mp", bufs=8) as mpool,
            tc.tile_pool(name="pp", bufs=8, space="PSUM") as ppool,
        ):
            mt_t = cpool.tile([P2, P2], f32, name="mt_t")
            nc.sync.dma_start(mt_t[:, :], mt[:, :])
            th_t = cpool.tile([P2, 4], f32, name="th_t")
            nc.sync.dma_start(th_t[:, :], th[:, :])

            pair_idx = 0
            for name, C, L in W_SPECS:
                npairs = C // (2 * W)
                for m in range(npairs):
                    c0, c2 = 2 * m * W, (2 * m + 2) * W
                    # one DMA per tile: [44, 2W] DRAM slab -> [88, W] SBUF
                    # (partition p = b*44 + j picks up row j of column-block b)
                    xt = xpool.tile([P2, W], f32, name="xt", tag="xt")
                    nc.gpsimd.dma_start(
                        xt[:, :],
                        xw[name][:, c0:c2].rearrange("j (b w) -> b j w", b=2))
                    rt = rpool.tile([P2, W], f32, name="rt", tag="rt")
                    nc.gpsimd.dma_start(
                        rt[:, :],
                        rw[name][:, c0:c2].rearrange("j (b w) -> b j w", b=2))
                    # mk <- (ri < thr_row)
                    mk = mpool.tile([P2, W], mybir.dt.uint8, name="mk", tag="mk")
                    nc.vector.tensor_scalar(
                        mk[:, :], rt[:, :], th_t[:, L:L + 1], None,
                        mybir.AluOpType.is_lt,
                    )
                    # per-bank psum chunks: 8 in flight across pairs
                    for g in range(W // MMN):
                        sl = slice(g * MMN, (g + 1) * MMN)
                        ps = ppool.tile([P2, MMN], f32, name="ps", tag="ps")
                        nc.tensor.matmul(
                            ps[:, :], mt_t[:, :], xt[:, sl],
                            start=True, stop=True,
                        )
                        nc.vector.copy_predicated(xt[:, sl], mk[:, sl], ps[:, :])
                    nc.gpsimd.dma_start(
                        ow[name][:, c0:c2].rearrange("j (b w) -> b j w", b=2),
                        xt[:, :])
                    pair_idx += 1

            # biases: one [44, 320] tile, per-layer column ranges 128|128|64
            xbt = xpool.tile([NP_POP, CB], f32, name="xbt", tag="xt")
            nc.gpsimd.dma_start(xbt[:, :], xb[:, :])
            rbt = rpool.tile([NP_POP, CB], f32, name="rbt", tag="rt")
            nc.gpsimd.dma_start(rbt[:, :], rb[:, :])
            psb = ppool.tile([NP_POP, CB], f32, name="psb", tag="ps")
            nc.tensor.matmul(
                psb[:, :], mt_t[0:NP_POP, 0:NP_POP], xbt[:, :],
                start=True, stop=True,
            )
            mkb = mpool.tile([NP_POP, CB], mybir.dt.uint8, name="mkb", tag="mk")
            bounds = (0, CB0, 2 * CB0, CB)
            for L in range(3):
                lo, hi = bounds[L], bounds[L + 1]
                nc.vector.tensor_scalar(
                    mkb[:, lo:hi], rbt[:, lo:hi], th_t[0:NP_POP, L:L + 1], None,
                    mybir.AluOpType.is_lt,
                )
            nc.vector.copy_predicated(xbt[:, :], mkb[:, :], psb[:, :])
            nc.gpsimd.dma_start(ob[:, :], xbt[:, :])

    if not nc.is_finalized():
        nc.finalize()
    _PROGRAM = nc
    return nc


def _host_side(fx, fy, Rs, a0, a1, best):
    """M^T (stacked block-diag) and the per-layer row thresholds."""
    f32 = np.float32
    idx = np.arange(NP_POP)
    M = np.zeros((NP_POP, NP_POP), np.float64)
    M[idx, idx] += 1.0 - F_CONST
    M[:, best] += F_CONST
    np.add.at(M, (idx, a0), F_CONST)
    np.add.at(M, (idx, a1), -F_CONST)
    MT = np.ascontiguousarray(M.T).astype(f32)
    mt_in = np.zeros((2 * NP_POP, 2 * NP_POP), f32)
    mt_in[:NP_POP, :NP_POP] = MT
    mt_in[NP_POP:, NP_POP:] = MT

    accept = fy <= fx
    th_in = np.zeros((2 * NP_POP, 4), f32)
    for L in range(3):
        thr = np.where(accept, np.where(Rs == L, f32(1.9), f32(CR_CONST)), f32(-1.0))
        th_in[:NP_POP, L] = thr
        th_in[NP_POP:, L] = thr
    return mt_in, th_in, accept


def kernel(w0, b0, w1, b1, w2, b2,
           ri_w0, ri_b0, ri_w1, ri_b1, ri_w2, ri_b2,
           fx, fy, best_model, a0, a1, Rs,
           _trace=False):
    f32 = np.float32
    w0 = np.asarray(w0, f32); b0 = np.asarray(b0, f32)
    w1 = np.asarray(w1, f32); b1 = np.asarray(b1, f32)
    w2 = np.asarray(w2, f32); b2 = np.asarray(b2, f32)
    ri_w0 = np.asarray(ri_w0, f32); ri_b0 = np.asarray(ri_b0, f32)
    ri_w1 = np.asarray(ri_w1, f32); ri_b1 = np.asarray(ri_b1, f32)
    ri_w2 = np.asarray(ri_w2, f32); ri_b2 = np.asarray(ri_b2, f32)
    fx = np.asarray(fx, f32); fy = np.asarray(fy, f32)
    a0 = np.asarray(a0, np.int64); a1 = np.asarray(a1, np.int64)
    Rs = np.asarray(Rs, np.int64)
    best = int(best_model)

    mt_in, th_in, accept = _host_side(fx, fy, Rs, a0, a1, best)

    wf = {"w0": w0.reshape(NP_POP, -1), "w1": w1.reshape(NP_POP, -1),
          "w2": w2.reshape(NP_POP, -1)}
    rf = {"w0": ri_w0.reshape(NP_POP, -1), "w1": ri_w1.reshape(NP_POP, -1),
          "w2": ri_w2.reshape(NP_POP, -1)}

    in_maps = []
    for k in range(N_CORES):
        im = {"mt": mt_in, "th": th_in}
        for name, C, _ in W_SPECS:
            im[f"x_{name}"] = np.ascontiguousarray(wf[name][:, k * C:(k + 1) * C])
            im[f"r_{name}"] = np.ascontiguousarray(rf[name][:, k * C:(k + 1) * C])
        im["x_b"] = np.ascontiguousarray(np.concatenate(
            [b0[:, k * CB0:(k + 1) * CB0], b1[:, k * CB0:(k + 1) * CB0],
             b2[:, k * CB2:(k + 1) * CB2]], axis=1))
        im["r_b"] = np.ascontiguousarray(np.concatenate(
            [ri_b0[:, k * CB0:(k + 1) * CB0], ri_b1[:, k * CB0:(k + 1) * CB0],
             ri_b2[:, k * CB2:(k + 1) * CB2]], axis=1))
        in_maps.append(im)

    nc = _build_program()
    res = run_bass_kernel_spmd(nc, in_maps, core_ids=list(range(N_CORES)),
                               trace=_trace)
    outs = res.results

    new_w0 = np.concatenate([outs[k]["o_w0"] for k in range(N_CORES)], axis=1) \
        .reshape(NP_POP, HID, IN_D)
    new_w1 = np.concatenate([outs[k]["o_w1"] for k in range(N_CORES)], axis=1) \
        .reshape(NP_POP, HID, HID)
    new_w2 = np.concatenate([outs[k]["o_w2"] for k in range(N_CORES)], axis=1) \
        .reshape(NP_POP, OUT_D, HID)
    new_b0 = np.concatenate([outs[k]["o_b"][:, 0:CB0] for k in range(N_CORES)], axis=1)
    new_b1 = np.concatenate([outs[k]["o_b"][:, CB0:2 * CB0] for k in range(N_CORES)], axis=1)
    new_b2 = np.concatenate([outs[k]["o_b"][:, 2 * CB0:CB] for k in range(N_CORES)], axis=1)

    new_fx = np.where(accept, fy, fx).astype(f32)
    min_f = np.float32(new_fx.min())
    best_out = np.int32(np.argmin(new_fx))

    out = (new_w0, new_b0, new_w1, new_b1, new_w2, new_b2, new_fx, min_f, best_out)
    if _trace:
        return out, res
    return out


# revision 14
# speedup vs baseline: 1.8204x; 1.8204x over previous
"""DE/NN population-update kernel for Trainium2 (8 NeuronCores).

Reference computation (per parameter tensor p with uniform tensor ri):
    mutant = p + F*(p[best] - p) + F*(p[a0] - p[a1])        (gathers along NP axis)
    cond   = (ri < CR) | (Rs == layer)[:, None...]
    y      = where(cond, mutant, p)
    out    = where((fy <= fx)[:, None...], y, p)

Key transforms:
  * mutant = M @ p along the NP=44 axis with
        M[i,j] = (1-F)*d_ij + F*d[j==best] + F*d[j==a0[i]] - F*d[j==a1[i]]
    so the row gathers become one 88x88 block-diag matmul (2 column-blocks
    stacked) on the otherwise idle TensorEngine -- no extra HBM traffic.
  * cond & accept fold into a per-row threshold on ri:
        thr_i = -1.0 (reject row) / 0.9 (CR test) / 1.9 (forced crossover)
    so per element:  mask = (ri < thr_row); out = mask ? mutant : p
  * Sharding: every [NP, ...] tensor is split along flattened trailing dims
    across 8 cores; fx/fy/min/argmin are 44-element host-side ops.
  * DMA layout: HW-measured on trn2, only flat 2-D 128-partition DMAs reach
    HBM roofline (~376 GB/s vs ~170-210 for 44/88-partition or 3-D-AP
    transfers). So the host prepacks each core's slab into [128, nst*W]:
    super-tile s holds column-blocks (2s, 2s+1) on partition rows 0-43 /
    44-87, rows 88-127 are pad. 45% extra bytes, ~2x effective bandwidth.
"""

import numpy as np

import concourse.bacc as bacc
import concourse.mybir as mybir
from concourse.bass_utils import run_bass_kernel_spmd
from concourse.tile import TileContext

F_CONST = 0.8
CR_CONST = 0.9
NP_POP = 44
IN_D, HID, OUT_D = 512, 1024, 512
N_CORES = 8

CW0 = HID * IN_D // N_CORES     # 65536 cols/core
CW1 = HID * HID // N_CORES      # 131072
CW2 = OUT_D * HID // N_CORES    # 65536
CB0 = HID // N_CORES            # 128
CB2 = OUT_D // N_CORES          # 64
CB = 2 * CB0 + CB2              # 320 (b0|b1|b2 concat)

W = 2048                        # columns per block; super-tile = 2 blocks
MMN = 512                       # fp32 matmul moving-operand max (1 PSUM bank)
P2 = 2 * NP_POP                 # 88 live partitions
PFULL = 128

# (name, cols-per-core, layer index) for the big weight tensors
W_SPECS = (("w0", CW0, 0), ("w1", CW1, 1), ("w2", CW2, 2))

_PROGRAM = None


def _nst(C):
    return C // (2 * W)


def _build_program():
    """Trace the per-core Bass/Tile program (shapes only; values are inputs)."""
    global _PROGRAM
    if _PROGRAM is not None:
        return _PROGRAM

    nc = bacc.Bacc()
    f32 = mybir.dt.float32
    u8 = mybir.dt.uint8

    px, pr, po = {}, {}, {}
    for name, C, _ in W_SPECS:
        n = _nst(C)
        px[name] = nc.dram_tensor(f"px_{name}", [PFULL, n * W], f32, kind="ExternalInput")
        pr[name] = nc.dram_tensor(f"pr_{name}", [PFULL, n * W], f32, kind="ExternalInput")
        po[name] = nc.dram_tensor(f"po_{name}", [PFULL, n * W], f32, kind="ExternalOutput")
    xb = nc.dram_tensor("x_b", [NP_POP, CB], f32, kind="ExternalInput")
    rb = nc.dram_tensor("r_b", [NP_POP, CB], f32, kind="ExternalInput")
    ob = nc.dram_tensor("o_b", [NP_POP, CB], f32, kind="ExternalOutput")
    mt = nc.dram_tensor("mt", [P2, P2], f32, kind="ExternalInput")   # block-diag(M^T, M^T)
    th = nc.dram_tensor("th", [P2, 4], f32, kind="ExternalInput")    # col L = [thr_L; thr_L]

    with TileContext(nc) as tc:
        with (
            tc.tile_pool(name="const", bufs=1) as cpool,
            tc.tile_pool(name="xp", bufs=8) as xpool,
            tc.tile_pool(name="rp", bufs=8) as rpool,
            tc.tile_pool(name="mp", bufs=8) as mpool,
            tc.tile_pool(name="pp", bufs=8, space="PSUM") as ppool,
        ):
            mt_t = cpool.tile([P2, P2], f32, name="mt_t")
            nc.sync.dma_start(mt_t[:, :], mt[:, :])
            th_t = cpool.tile([P2, 4], f32, name="th_t")
            nc.sync.dma_start(th_t[:, :], th[:, :])

            for name, C, L in W_SPECS:
                for s in range(_nst(C)):
                    sl0 = slice(s * W, (s + 1) * W)
                    xt = xpool.tile([PFULL, W], f32, name="xt", tag="xt")
                    nc.gpsimd.dma_start(xt[:, :], px[name][:, sl0])
                    rt = rpool.tile([PFULL, W], f32, name="rt", tag="rt")
                    nc.gpsimd.dma_start(rt[:, :], pr[name][:, sl0])
                    mk = mpool.tile([P2, W], u8, name="mk", tag="mk")
                    nc.vector.tensor_scalar(
                        mk[:, :], rt[0:P2, :], th_t[:, L:L + 1], None,
                        mybir.AluOpType.is_lt,
                    )
                    for g in range(W // MMN):
                        sl = slice(g * MMN, (g + 1) * MMN)
                        ps = ppool.tile([P2, MMN], f32, name="ps", tag="ps")
                        nc.tensor.matmul(
                            ps[:, :], mt_t[:, :], xt[0:P2, sl],
                            start=True, stop=True,
                        )
                        nc.vector.copy_predicated(xt[0:P2, sl], mk[:, sl], ps[:, :])
                    nc.gpsimd.dma_start(po[name][:, sl0], xt[:, :])

            # biases: one [44, 320] tile, per-layer column ranges 128|128|64
            xbt = xpool.tile([NP_POP, CB], f32, name="xbt", tag="xt")
            nc.gpsimd.dma_start(xbt[:, :], xb[:, :])
            rbt = rpool.tile([NP_POP, CB], f32, name="rbt", tag="rt")
            nc.gpsimd.dma_start(rbt[:, :], rb[:, :])
            psb = ppool.tile([NP_POP, CB], f32, name="psb", tag="ps")
            nc.tensor.matmul(
                psb[:, :], mt_t[0:NP_POP, 0:NP_POP], xbt[:, :],
                start=True, stop=True,
            )
            mkb = mpool.tile([NP_POP, CB], u8, name="mkb", tag="mk")
            bounds = (0, CB0, 2 * CB0, CB)
            for L in range(3):
                lo, hi = bounds[L], bounds[L + 1]
                nc.vector.tensor_scalar(
                    mkb[:, lo:hi], rbt[:, lo:hi], th_t[0:NP_POP, L:L + 1], None,
                    mybir.AluOpType.is_lt,
                )
            nc.vector.copy_predicated(xbt[:, :], mkb[:, :], psb[:, :])
            nc.gpsimd.dma_start(ob[:, :], xbt[:, :])

    if not nc.is_finalized():
        nc.finalize()
    _PROGRAM = nc
    return nc


def _pack(slab):
    """[44, C] core slab -> [128, nst*W] super-tile layout (rows 88+ pad)."""
    C = slab.shape[1]
    n = _nst(C)
    v = slab.reshape(NP_POP, n, 2, W)
    pack = np.empty((PFULL, n, W), np.float32)
    pack[0:NP_POP] = v[:, :, 0, :]
    pack[NP_POP:P2] = v[:, :, 1, :]
    pack[P2:] = 0.0
    return pack.reshape(PFULL, n * W)


def _unpack(packed, C):
    """[128, nst*W] -> [44, C]."""
    n = _nst(C)
    o3 = packed.reshape(PFULL, n, W)
    out = np.empty((NP_POP, C), np.float32)
    v = out.reshape(NP_POP, n, 2, W)
    v[:, :, 0, :] = o3[0:NP_POP]
    v[:, :, 1, :] = o3[NP_POP:P2]
    return out


def _host_side(fx, fy, Rs, a0, a1, best):
    """M^T (stacked block-diag) and the per-layer row thresholds."""
    f32 = np.float32
    idx = np.arange(NP_POP)
    M = np.zeros((NP_POP, NP_POP), np.float64)
    M[idx, idx] += 1.0 - F_CONST
    M[:, best] += F_CONST
    np.add.at(M, (idx, a0), F_CONST)
    np.add.at(M, (idx, a1), -F_CONST)
    MT = np.ascontiguousarray(M.T).astype(f32)
    mt_in = np.zeros((P2, P2), f32)
    mt_in[:NP_POP, :NP_POP] = MT
    mt_in[NP_POP:, NP_POP:] = MT

    accept = fy <= fx
    th_in = np.zeros((P2, 4), f32)
    for L in range(3):
        thr = np.where(accept, np.where(Rs == L, f32(1.9), f32(CR_CONST)), f32(-1.0))
        th_in[:NP_POP, L] = thr
        th_in[NP_POP:, L] = thr
    return mt_in, th_in, accept


def kernel(w0, b0, w1, b1, w2, b2,
           ri_w0, ri_b0, ri_w1, ri_b1, ri_w2, ri_b2,
           fx, fy, best_model, a0, a1, Rs,
           _trace=False):
    f32 = np.float32
    w0 = np.asarray(w0, f32); b0 = np.asarray(b0, f32)
    w1 = np.asarray(w1, f32); b1 = np.asarray(b1, f32)
    w2 = np.asarray(w2, f32); b2 = np.asarray(b2, f32)
    ri_w0 = np.asarray(ri_w0, f32); ri_b0 = np.asarray(ri_b0, f32)
    ri_w1 = np.asarray(ri_w1, f32); ri_b1 = np.asarray(ri_b1, f32)
    ri_w2 = np.asarray(ri_w2, f32); ri_b2 = np.asarray(ri_b2, f32)
    fx = np.asarray(fx, f32); fy = np.asarray(fy, f32)
    a0 = np.asarray(a0, np.int64); a1 = np.asarray(a1, np.int64)
    Rs = np.asarray(Rs, np.int64)
    best = int(best_model)

    mt_in, th_in, accept = _host_side(fx, fy, Rs, a0, a1, best)

    wf = {"w0": w0.reshape(NP_POP, -1), "w1": w1.reshape(NP_POP, -1),
          "w2": w2.reshape(NP_POP, -1)}
    rf = {"w0": ri_w0.reshape(NP_POP, -1), "w1": ri_w1.reshape(NP_POP, -1),
          "w2": ri_w2.reshape(NP_POP, -1)}

    in_maps = []
    for k in range(N_CORES):
        im = {"mt": mt_in, "th": th_in}
        for name, C, _ in W_SPECS:
            im[f"px_{name}"] = _pack(wf[name][:, k * C:(k + 1) * C])
            im[f"pr_{name}"] = _pack(rf[name][:, k * C:(k + 1) * C])
        im["x_b"] = np.ascontiguousarray(np.concatenate(
            [b0[:, k * CB0:(k + 1) * CB0], b1[:, k * CB0:(k + 1) * CB0],
             b2[:, k * CB2:(k + 1) * CB2]], axis=1))
        im["r_b"] = np.ascontiguousarray(np.concatenate(
            [ri_b0[:, k * CB0:(k + 1) * CB0], ri_b1[:, k * CB0:(k + 1) * CB0],
             ri_b2[:, k * CB2:(k + 1) * CB2]], axis=1))
        in_maps.append(im)

    nc = _build_program()
    res = run_bass_kernel_spmd(nc, in_maps, core_ids=list(range(N_CORES)),
                               trace=_trace)
    outs = res.results

    def gather(name, C):
        return np.concatenate(
            [_unpack(outs[k][f"po_{name}"], C) for k in range(N_CORES)], axis=1)

    new_w0 = gather("w0", CW0).reshape(NP_POP, HID, IN_D)
    new_w1 = gather("w1", CW1).reshape(NP_POP, HID, HID)
    new_w2 = gather("w2", CW2).reshape(NP_POP, OUT_D, HID)
    new_b0 = np.concatenate([outs[k]["o_b"][:, 0:CB0] for k in range(N_CORES)], axis=1)
    new_b1 = np.concatenate([outs[k]["o_b"][:, CB0:2 * CB0] for k in range(N_CORES)], axis=1)
    new_b2 = np.concatenate([outs[k]["o_b"][:, 2 * CB0:CB] for k in range(N_CORES)], axis=1)

    new_fx = np.where(accept, fy, fx).astype(f32)
    min_f = np.float32(new_fx.min())
    best_out = np.int32(np.argmin(new_fx))

    out = (new_w0, new_b0, new_w1, new_b1, new_w2, new_b2, new_fx, min_f, best_out)
    if _trace:
        return out, res
    return out


# revision 18
# speedup vs baseline: 2.2099x; 1.2139x over previous
"""DE/NN population-update kernel for Trainium2 (8 NeuronCores).

Reference computation (per parameter tensor p with uniform tensor ri):
    mutant = p + F*(p[best] - p) + F*(p[a0] - p[a1])        (gathers along NP axis)
    cond   = (ri < CR) | (Rs == layer)[:, None...]
    y      = where(cond, mutant, p)
    out    = where((fy <= fx)[:, None...], y, p)

Key transforms:
  * mutant = M @ p along the NP=44 axis with
        M[i,j] = (1-F)*d_ij + F*d[j==best] + F*d[j==a0[i]] - F*d[j==a1[i]]
    so the row gathers become one 88x88 block-diag matmul (2 column-blocks
    stacked) on the otherwise idle TensorEngine -- no extra HBM traffic.
  * cond & accept fold into a per-row threshold on ri:
        thr_i = -1.0 (reject row) / 0.9 (CR test) / 1.9 (forced crossover)
    so per element:  mask = (ri < thr_row); out = mask ? mutant : p
  * Sharding: every [NP, ...] tensor is split along flattened trailing dims
    across 8 cores; fx/fy/min/argmin are 44-element host-side ops.
  * DMA layout: HW-measured on trn2, only flat 2-D 128-partition DMAs reach
    HBM roofline (~376 GB/s vs ~170-210 for 44/88-partition or 3-D-AP
    transfers). So the host prepacks each core's slab into [128, nst*W]:
    super-tile s holds column-blocks (2s, 2s+1) on partition rows 0-43 /
    44-87, rows 88-127 are pad. 45% extra bytes, ~2x effective bandwidth.
"""

import numpy as np

import concourse.bacc as bacc
import concourse.mybir as mybir
from concourse.bass_utils import run_bass_kernel_spmd
from concourse.tile import TileContext

F_CONST = 0.8
CR_CONST = 0.9
NP_POP = 44
IN_D, HID, OUT_D = 512, 1024, 512
N_CORES = 8

CW0 = HID * IN_D // N_CORES     # 65536 cols/core
CW1 = HID * HID // N_CORES      # 131072
CW2 = OUT_D * HID // N_CORES    # 65536
CB0 = HID // N_CORES            # 128
CB2 = OUT_D // N_CORES          # 64
CB = 2 * CB0 + CB2              # 320 (b0|b1|b2 concat)

W = 2048                        # columns per block; super-tile = 2 blocks
MMN = 512                       # fp32 matmul moving-operand max (1 PSUM bank)
P2 = 2 * NP_POP                 # 88 live partitions
PFULL = 128

# (name, cols-per-core, layer index) for the big weight tensors
W_SPECS = (("w0", CW0, 0), ("w1", CW1, 1), ("w2", CW2, 2))

_PROGRAM = None


def _nst(C):
    return C // (2 * W)


def _build_program():
    """Trace the per-core Bass/Tile program (shapes only; values are inputs)."""
    global _PROGRAM
    if _PROGRAM is not None:
        return _PROGRAM

    nc = bacc.Bacc()
    f32 = mybir.dt.float32
    u8 = mybir.dt.uint8

    px, pr, po = {}, {}, {}
    for name, C, _ in W_SPECS:
        n = _nst(C)
        px[name] = nc.dram_tensor(f"px_{name}", [PFULL, n * W], f32, kind="ExternalInput")
        # ri rides densely: 3 x 32-row slots per super-tile (32|32|24+8pad)
        pr[name] = nc.dram_tensor(f"pr_{name}", [PFULL, (3 * n // 4) * W], f32,
                                  kind="ExternalInput")
        po[name] = nc.dram_tensor(f"po_{name}", [PFULL, n * W], f32, kind="ExternalOutput")
    xb = nc.dram_tensor("x_b", [NP_POP, CB], f32, kind="ExternalInput")
    rb = nc.dram_tensor("r_b", [NP_POP, CB], f32, kind="ExternalInput")
    ob = nc.dram_tensor("o_b", [NP_POP, CB], f32, kind="ExternalOutput")
    mt = nc.dram_tensor("mt", [P2, P2], f32, kind="ExternalInput")   # block-diag(M^T, M^T)
    th = nc.dram_tensor("th", [P2, 4], f32, kind="ExternalInput")    # col L = [thr_L; thr_L]

    with TileContext(nc) as tc:
        with (
            tc.tile_pool(name="const", bufs=1) as cpool,
            tc.tile_pool(name="xp", bufs=9) as xpool,
            tc.tile_pool(name="rp", bufs=8) as rpool,
            tc.tile_pool(name="mp", bufs=8) as mpool,
            tc.tile_pool(name="pp", bufs=8, space="PSUM") as ppool,
        ):
            mt_t = cpool.tile([P2, P2], f32, name="mt_t")
            nc.sync.dma_start(mt_t[:, :], mt[:, :])
            th_t = cpool.tile([P2, 4], f32, name="th_t")
            nc.sync.dma_start(th_t[:, :], th[:, :])

            for name, C, L in W_SPECS:
                rslices = {}

                def rslice(b, name=name):
                    if b not in rslices:
                        rt = rpool.tile([PFULL, W], f32, name="rt", tag="rt")
                        nc.gpsimd.dma_start(rt[:, :], pr[name][:, b * W:(b + 1) * W])
                        rslices[b] = rt
                    return rslices[b]

                for s in range(_nst(C)):
                    sl0 = slice(s * W, (s + 1) * W)
                    xt = xpool.tile([PFULL, W], f32, name="xt", tag="xt")
                    nc.gpsimd.dma_start(xt[:, :], px[name][:, sl0])
                    mk = mpool.tile([P2, W], u8, name="mk", tag="mk")
                    for q, rows in ((0, 32), (1, 32), (2, 24)):
                        t = 3 * s + q
                        rt = rslice(t // 4)
                        ro = 32 * (t % 4)
                        mo = 32 * q
                        nc.vector.tensor_scalar(
                            mk[mo:mo + rows, :], rt[ro:ro + rows, :],
                            th_t[mo:mo + rows, L:L + 1], None,
                            mybir.AluOpType.is_lt,
                        )
                    for g in range(W // MMN):
                        sl = slice(g * MMN, (g + 1) * MMN)
                        ps = ppool.tile([P2, MMN], f32, name="ps", tag="ps")
                        nc.tensor.matmul(
                            ps[:, :], mt_t[:, :], xt[0:P2, sl],
                            start=True, stop=True,
                        )
                        nc.vector.copy_predicated(xt[0:P2, sl], mk[:, sl], ps[:, :])
                    nc.gpsimd.dma_start(po[name][:, sl0], xt[:, :])

            # biases: one [44, 320] tile, per-layer column ranges 128|128|64
            xbt = xpool.tile([NP_POP, CB], f32, name="xbt", tag="xt")
            nc.gpsimd.dma_start(xbt[:, :], xb[:, :])
            rbt = rpool.tile([NP_POP, CB], f32, name="rbt", tag="rt")
            nc.gpsimd.dma_start(rbt[:, :], rb[:, :])
            psb = ppool.tile([NP_POP, CB], f32, name="psb", tag="ps")
            nc.tensor.matmul(
                psb[:, :], mt_t[0:NP_POP, 0:NP_POP], xbt[:, :],
                start=True, stop=True,
            )
            mkb = mpool.tile([NP_POP, CB], u8, name="mkb", tag="mk")
            bounds = (0, CB0, 2 * CB0, CB)
            for L in range(3):
                lo, hi = bounds[L], bounds[L + 1]
                nc.vector.tensor_scalar(
                    mkb[:, lo:hi], rbt[:, lo:hi], th_t[0:NP_POP, L:L + 1], None,
                    mybir.AluOpType.is_lt,
                )
            nc.vector.copy_predicated(xbt[:, :], mkb[:, :], psb[:, :])
            nc.gpsimd.dma_start(ob[:, :], xbt[:, :])

    if not nc.is_finalized():
        nc.finalize()
    _PROGRAM = nc
    return nc


def _pack(slab):
    """[44, C] core slab -> [128, nst*W] super-tile layout (rows 88+ pad)."""
    C = slab.shape[1]
    n = _nst(C)
    v = slab.reshape(NP_POP, n, 2, W)
    pack = np.empty((PFULL, n, W), np.float32)
    pack[0:NP_POP] = v[:, :, 0, :]
    pack[NP_POP:P2] = v[:, :, 1, :]
    pack[P2:] = 0.0
    return pack.reshape(PFULL, n * W)


def _pack_ri(slab):
    """[44, C] ri slab -> dense [128, (3n/4)*W]: 32-row slots, 3 per tile."""
    C = slab.shape[1]
    n = _nst(C)
    v = slab.reshape(NP_POP, n, 2, W)
    prow = np.empty((P2, n, W), np.float32)
    prow[0:NP_POP] = v[:, :, 0, :]
    prow[NP_POP:P2] = v[:, :, 1, :]
    # pieces [3n, 32, W]: piece 3s+q = pair-rows [32q, 32q+32) of tile s
    P = np.zeros((3 * n, 32, W), np.float32)
    P[0::3] = prow[0:32].transpose(1, 0, 2)
    P[1::3] = prow[32:64].transpose(1, 0, 2)
    P[2::3, 0:24] = prow[64:88].transpose(1, 0, 2)
    # slot t -> (partition 32*(t%4), column-block t//4)
    return np.ascontiguousarray(
        P.reshape(3 * n // 4, 4, 32, W).transpose(1, 2, 0, 3)
    ).reshape(PFULL, (3 * n // 4) * W)


def _unpack(packed, C):
    """[128, nst*W] -> [44, C]."""
    n = _nst(C)
    o3 = packed.reshape(PFULL, n, W)
    out = np.empty((NP_POP, C), np.float32)
    v = out.reshape(NP_POP, n, 2, W)
    v[:, :, 0, :] = o3[0:NP_POP]
    v[:, :, 1, :] = o3[NP_POP:P2]
    return out


def _host_side(fx, fy, Rs, a0, a1, best):
    """M^T (stacked block-diag) and the per-layer row thresholds."""
    f32 = np.float32
    idx = np.arange(NP_POP)
    M = np.zeros((NP_POP, NP_POP), np.float64)
    M[idx, idx] += 1.0 - F_CONST
    M[:, best] += F_CONST
    np.add.at(M, (idx, a0), F_CONST)
    np.add.at(M, (idx, a1), -F_CONST)
    MT = np.ascontiguousarray(M.T).astype(f32)
    mt_in = np.zeros((P2, P2), f32)
    mt_in[:NP_POP, :NP_POP] = MT
    mt_in[NP_POP:, NP_POP:] = MT

    accept = fy <= fx
    th_in = np.zeros((P2, 4), f32)
    for L in range(3):
        thr = np.where(accept, np.where(Rs == L, f32(1.9), f32(CR_CONST)), f32(-1.0))
        th_in[:NP_POP, L] = thr
        th_in[NP_POP:, L] = thr
    return mt_in, th_in, accept


def kernel(w0, b0, w1, b1, w2, b2,
           ri_w0, ri_b0, ri_w1, ri_b1, ri_w2, ri_b2,
           fx, fy, best_model, a0, a1, Rs,
           _trace=False):
    f32 = np.float32
    w0 = np.asarray(w0, f32); b0 = np.asarray(b0, f32)
    w1 = np.asarray(w1, f32); b1 = np.asarray(b1, f32)
    w2 = np.asarray(w2, f32); b2 = np.asarray(b2, f32)
    ri_w0 = np.asarray(ri_w0, f32); ri_b0 = np.asarray(ri_b0, f32)
    ri_w1 = np.asarray(ri_w1, f32); ri_b1 = np.asarray(ri_b1, f32)
    ri_w2 = np.asarray(ri_w2, f32); ri_b2 = np.asarray(ri_b2, f32)
    fx = np.asarray(fx, f32); fy = np.asarray(fy, f32)
    a0 = np.asarray(a0, np.int64); a1 = np.asarray(a1, np.int64)
    Rs = np.asarray(Rs, np.int64)
    best = int(best_model)

    mt_in, th_in, accept = _host_side(fx, fy, Rs, a0, a1, best)

    wf = {"w0": w0.reshape(NP_POP, -1), "w1": w1.reshape(NP_POP, -1),
          "w2": w2.reshape(NP_POP, -1)}
    rf = {"w0": ri_w0.reshape(NP_POP, -1), "w1": ri_w1.reshape(NP_POP, -1),
          "w2": ri_w2.reshape(NP_POP, -1)}

    in_maps = []
    for k in range(N_CORES):
        im = {"mt": mt_in, "th": th_in}
        for name, C, _ in W_SPECS:
            im[f"px_{name}"] = _pack(wf[name][:, k * C:(k + 1) * C])
            im[f"pr_{name}"] = _pack_ri(rf[name][:, k * C:(k + 1) * C])
        im["x_b"] = np.ascontiguousarray(np.concatenate(
            [b0[:, k * CB0:(k + 1) * CB0], b1[:, k * CB0:(k + 1) * CB0],
             b2[:, k * CB2:(k + 1) * CB2]], axis=1))
        im["r_b"] = np.ascontiguousarray(np.concatenate(
            [ri_b0[:, k * CB0:(k + 1) * CB0], ri_b1[:, k * CB0:(k + 1) * CB0],
             ri_b2[:, k * CB2:(k + 1) * CB2]], axis=1))
        in_maps.append(im)

    nc = _build_program()
    res = run_bass_kernel_spmd(nc, in_maps, core_ids=list(range(N_CORES)),
                               trace=_trace)
    outs = res.results

    def gather(name, C):
        return np.concatenate(
            [_unpack(outs[k][f"po_{name}"], C) for k in range(N_CORES)], axis=1)

    new_w0 = gather("w0", CW0).reshape(NP_POP, HID, IN_D)
    new_w1 = gather("w1", CW1).reshape(NP_POP, HID, HID)
    new_w2 = gather("w2", CW2).reshape(NP_POP, OUT_D, HID)
    new_b0 = np.concatenate([outs[k]["o_b"][:, 0:CB0] for k in range(N_CORES)], axis=1)
    new_b1 = np.concatenate([outs[k]["o_b"][:, CB0:2 * CB0] for k in range(N_CORES)], axis=1)
    new_b2 = np.concatenate([outs[k]["o_b"][:, 2 * CB0:CB] for k in range(N_CORES)], axis=1)

    new_fx = np.where(accept, fy, fx).astype(f32)
    min_f = np.float32(new_fx.min())
    best_out = np.int32(np.argmin(new_fx))

    out = (new_w0, new_b0, new_w1, new_b1, new_w2, new_b2, new_fx, min_f, best_out)
    if _trace:
        return out, res
    return out


# revision 22
# speedup vs baseline: 2.2258x; 1.0072x over previous
"""DE/NN population-update kernel for Trainium2 (8 NeuronCores).

Reference computation (per parameter tensor p with uniform tensor ri):
    mutant = p + F*(p[best] - p) + F*(p[a0] - p[a1])        (gathers along NP axis)
    cond   = (ri < CR) | (Rs == layer)[:, None...]
    y      = where(cond, mutant, p)
    out    = where((fy <= fx)[:, None...], y, p)

Key transforms:
  * mutant = M @ p along the NP=44 axis with
        M[i,j] = (1-F)*d_ij + F*d[j==best] + F*d[j==a0[i]] - F*d[j==a1[i]]
    so the row gathers become one 88x88 block-diag matmul (2 column-blocks
    stacked) on the otherwise idle TensorEngine -- no extra HBM traffic.
  * cond & accept fold into a per-row threshold on ri:
        thr_i = -1.0 (reject row) / 0.9 (CR test) / 1.9 (forced crossover)
    so per element:  mask = (ri < thr_row); out = mask ? mutant : p
  * Sharding: every [NP, ...] tensor is split along flattened trailing dims
    across 8 cores; fx/fy/min/argmin are 44-element host-side ops.
  * DMA layout: HW-measured on trn2, only flat 2-D 128-partition DMAs reach
    HBM roofline (~376 GB/s vs ~170-210 for 44/88-partition or 3-D-AP
    transfers). So the host prepacks each core's slab into [128, nst*W]:
    super-tile s holds column-blocks (2s, 2s+1) on partition rows 0-43 /
    44-87, rows 88-127 are pad. 45% extra bytes, ~2x effective bandwidth.
"""

import numpy as np

import concourse.bacc as bacc
import concourse.mybir as mybir
from concourse.bass_utils import run_bass_kernel_spmd
from concourse.tile import TileContext

F_CONST = 0.8
CR_CONST = 0.9
NP_POP = 44
IN_D, HID, OUT_D = 512, 1024, 512
N_CORES = 8

CW0 = HID * IN_D // N_CORES     # 65536 cols/core
CW1 = HID * HID // N_CORES      # 131072
CW2 = OUT_D * HID // N_CORES    # 65536
CB0 = HID // N_CORES            # 128
CB2 = OUT_D // N_CORES          # 64
CB = 2 * CB0 + CB2              # 320 (b0|b1|b2 concat)

W = 2048                        # columns per block; super-tile = 2 blocks
MMN = 512                       # fp32 matmul moving-operand max (1 PSUM bank)
P2 = 2 * NP_POP                 # 88 live partitions
PFULL = 128

# (name, cols-per-core, layer index) for the big weight tensors
W_SPECS = (("w0", CW0, 0), ("w1", CW1, 1), ("w2", CW2, 2))

_PROGRAM = None


def _nst(C):
    return C // (2 * W)


def _build_program():
    """Trace the per-core Bass/Tile program (shapes only; values are inputs)."""
    global _PROGRAM
    if _PROGRAM is not None:
        return _PROGRAM

    nc = bacc.Bacc()
    f32 = mybir.dt.float32
    u8 = mybir.dt.uint8

    px, pr, po = {}, {}, {}
    for name, C, _ in W_SPECS:
        n = _nst(C)
        px[name] = nc.dram_tensor(f"px_{name}", [PFULL, n * W], f32, kind="ExternalInput")
        # ri pieces q=0,1 (32 rows each) ride densely in pr; the 24-row q=2
        # piece rides free in the x-tile pad slot (rows 96-119)
        pr[name] = nc.dram_tensor(f"pr_{name}", [PFULL, (n // 2) * W], f32,
                                  kind="ExternalInput")
        po[name] = nc.dram_tensor(f"po_{name}", [PFULL, n * W], f32, kind="ExternalOutput")
    xb = nc.dram_tensor("x_b", [NP_POP, CB], f32, kind="ExternalInput")
    rb = nc.dram_tensor("r_b", [NP_POP, CB], f32, kind="ExternalInput")
    ob = nc.dram_tensor("o_b", [NP_POP, CB], f32, kind="ExternalOutput")
    mt = nc.dram_tensor("mt", [P2, P2], f32, kind="ExternalInput")   # block-diag(M^T, M^T)
    th = nc.dram_tensor("th", [P2, 4], f32, kind="ExternalInput")    # col L = [thr_L; thr_L]

    with TileContext(nc) as tc:
        with (
            tc.tile_pool(name="const", bufs=1) as cpool,
            tc.tile_pool(name="xp", bufs=9) as xpool,
            tc.tile_pool(name="rp", bufs=8) as rpool,
            tc.tile_pool(name="mp", bufs=8) as mpool,
            tc.tile_pool(name="pp", bufs=8, space="PSUM") as ppool,
        ):
            mt_t = cpool.tile([P2, P2], f32, name="mt_t")
            nc.sync.dma_start(mt_t[:, :], mt[:, :])
            th_t = cpool.tile([P2, 4], f32, name="th_t")
            nc.sync.dma_start(th_t[:, :], th[:, :])

            for name, C, L in W_SPECS:
                rslices = {}

                def rslice(b, name=name):
                    if b not in rslices:
                        rt = rpool.tile([PFULL, W], f32, name="rt", tag="rt")
                        nc.gpsimd.dma_start(rt[:, :], pr[name][:, b * W:(b + 1) * W])
                        rslices[b] = rt
                    return rslices[b]

                for s in range(_nst(C)):
                    sl0 = slice(s * W, (s + 1) * W)
                    xt = xpool.tile([PFULL, W], f32, name="xt", tag="xt")
                    nc.gpsimd.dma_start(xt[:, :], px[name][:, sl0])
                    mk = mpool.tile([P2, W], u8, name="mk", tag="mk")
                    for q in (0, 1):
                        t = 2 * s + q
                        rt = rslice(t // 4)
                        ro = 32 * (t % 4)
                        mo = 32 * q
                        nc.vector.tensor_scalar(
                            mk[mo:mo + 32, :], rt[ro:ro + 32, :],
                            th_t[mo:mo + 32, L:L + 1], None,
                            mybir.AluOpType.is_lt,
                        )
                    nc.vector.tensor_scalar(
                        mk[64:88, :], xt[96:120, :],
                        th_t[64:88, L:L + 1], None,
                        mybir.AluOpType.is_lt,
                    )
                    for g in range(W // MMN):
                        sl = slice(g * MMN, (g + 1) * MMN)
                        ps = ppool.tile([P2, MMN], f32, name="ps", tag="ps")
                        nc.tensor.matmul(
                            ps[:, :], mt_t[:, :], xt[0:P2, sl],
                            start=True, stop=True,
                        )
                        nc.vector.copy_predicated(xt[0:P2, sl], mk[:, sl], ps[:, :])
                    nc.gpsimd.dma_start(po[name][:, sl0], xt[:, :])

            # biases: one [44, 320] tile, per-layer column ranges 128|128|64
            xbt = xpool.tile([NP_POP, CB], f32, name="xbt", tag="xt")
            nc.gpsimd.dma_start(xbt[:, :], xb[:, :])
            rbt = rpool.tile([NP_POP, CB], f32, name="rbt", tag="rt")
            nc.gpsimd.dma_start(rbt[:, :], rb[:, :])
            psb = ppool.tile([NP_POP, CB], f32, name="psb", tag="ps")
            nc.tensor.matmul(
                psb[:, :], mt_t[0:NP_POP, 0:NP_POP], xbt[:, :],
                start=True, stop=True,
            )
            mkb = mpool.tile([NP_POP, CB], u8, name="mkb", tag="mk")
            bounds = (0, CB0, 2 * CB0, CB)
            for L in range(3):
                lo, hi = bounds[L], bounds[L + 1]
                nc.vector.tensor_scalar(
                    mkb[:, lo:hi], rbt[:, lo:hi], th_t[0:NP_POP, L:L + 1], None,
                    mybir.AluOpType.is_lt,
                )
            nc.vector.copy_predicated(xbt[:, :], mkb[:, :], psb[:, :])
            nc.gpsimd.dma_start(ob[:, :], xbt[:, :])

    if not nc.is_finalized():
        nc.finalize()
    _PROGRAM = nc
    return nc


def _pack(slab, ri_slab):
    """[44, C] core slab -> [128, nst*W] super-tile layout: rows 0-87 = two
    column-blocks of x, rows 96-119 = ri pair-rows 64-87 (the q=2 piece)."""
    C = slab.shape[1]
    n = _nst(C)
    v = slab.reshape(NP_POP, n, 2, W)
    rv = ri_slab.reshape(NP_POP, n, 2, W)
    pack = np.zeros((PFULL, n, W), np.float32)
    pack[0:NP_POP] = v[:, :, 0, :]
    pack[NP_POP:P2] = v[:, :, 1, :]
    # ri pair-rows 64-87 = ri rows 20-43 of the second column-block
    pack[96:120] = rv[20:44, :, 1, :]
    return pack.reshape(PFULL, n * W)


def _pack_ri(slab):
    """[44, C] ri slab -> dense [128, (n/2)*W]: q=0,1 pieces in 32-row slots."""
    C = slab.shape[1]
    n = _nst(C)
    v = slab.reshape(NP_POP, n, 2, W)
    prow = np.empty((P2, n, W), np.float32)
    prow[0:NP_POP] = v[:, :, 0, :]
    prow[NP_POP:P2] = v[:, :, 1, :]
    # pieces [2n, 32, W]: piece 2s+q = pair-rows [32q, 32q+32) of tile s
    P = np.empty((2 * n, 32, W), np.float32)
    P[0::2] = prow[0:32].transpose(1, 0, 2)
    P[1::2] = prow[32:64].transpose(1, 0, 2)
    # slot t -> (partition 32*(t%4), column-block t//4)
    return np.ascontiguousarray(
        P.reshape(n // 2, 4, 32, W).transpose(1, 2, 0, 3)
    ).reshape(PFULL, (n // 2) * W)


def _unpack(packed, C):
    """[128, nst*W] -> [44, C]."""
    n = _nst(C)
    o3 = packed.reshape(PFULL, n, W)
    out = np.empty((NP_POP, C), np.float32)
    v = out.reshape(NP_POP, n, 2, W)
    v[:, :, 0, :] = o3[0:NP_POP]
    v[:, :, 1, :] = o3[NP_POP:P2]
    return out


def _host_side(fx, fy, Rs, a0, a1, best):
    """M^T (stacked block-diag) and the per-layer row thresholds."""
    f32 = np.float32
    idx = np.arange(NP_POP)
    M = np.zeros((NP_POP, NP_POP), np.float64)
    M[idx, idx] += 1.0 - F_CONST
    M[:, best] += F_CONST
    np.add.at(M, (idx, a0), F_CONST)
    np.add.at(M, (idx, a1), -F_CONST)
    MT = np.ascontiguousarray(M.T).astype(f32)
    mt_in = np.zeros((P2, P2), f32)
    mt_in[:NP_POP, :NP_POP] = MT
    mt_in[NP_POP:, NP_POP:] = MT

    accept = fy <= fx
    th_in = np.zeros((P2, 4), f32)
    for L in range(3):
        thr = np.where(accept, np.where(Rs == L, f32(1.9), f32(CR_CONST)), f32(-1.0))
        th_in[:NP_POP, L] = thr
        th_in[NP_POP:, L] = thr
    return mt_in, th_in, accept


def kernel(w0, b0, w1, b1, w2, b2,
           ri_w0, ri_b0, ri_w1, ri_b1, ri_w2, ri_b2,
           fx, fy, best_model, a0, a1, Rs,
           _trace=False):
    f32 = np.float32
    w0 = np.asarray(w0, f32); b0 = np.asarray(b0, f32)
    w1 = np.asarray(w1, f32); b1 = np.asarray(b1, f32)
    w2 = np.asarray(w2, f32); b2 = np.asarray(b2, f32)
    ri_w0 = np.asarray(ri_w0, f32); ri_b0 = np.asarray(ri_b0, f32)
    ri_w1 = np.asarray(ri_w1, f32); ri_b1 = np.asarray(ri_b1, f32)
    ri_w2 = np.asarray(ri_w2, f32); ri_b2 = np.asarray(ri_b2, f32)
    fx = np.asarray(fx, f32); fy = np.asarray(fy, f32)
    a0 = np.asarray(a0, np.int64); a1 = np.asarray(a1, np.int64)
    Rs = np.asarray(Rs, np.int64)
    best = int(best_model)

    mt_in, th_in, accept = _host_side(fx, fy, Rs, a0, a1, best)

    wf = {"w0": w0.reshape(NP_POP, -1), "w1": w1.reshape(NP_POP, -1),
          "w2": w2.reshape(NP_POP, -1)}
    rf = {"w0": ri_w0.reshape(NP_POP, -1), "w1": ri_w1.reshape(NP_POP, -1),
          "w2": ri_w2.reshape(NP_POP, -1)}

    in_maps = []
    for k in range(N_CORES):
        im = {"mt": mt_in, "th": th_in}
        for name, C, _ in W_SPECS:
            im[f"px_{name}"] = _pack(wf[name][:, k * C:(k + 1) * C],
                                     rf[name][:, k * C:(k + 1) * C])
            im[f"pr_{name}"] = _pack_ri(rf[name][:, k * C:(k + 1) * C])
        im["x_b"] = np.ascontiguousarray(np.concatenate(
            [b0[:, k * CB0:(k + 1) * CB0], b1[:, k * CB0:(k + 1) * CB0],
             b2[:, k * CB2:(k + 1) * CB2]], axis=1))
        im["r_b"] = np.ascontiguousarray(np.concatenate(
            [ri_b0[:, k * CB0:(k + 1) * CB0], ri_b1[:, k * CB0:(k + 1) * CB0],
             ri_b2[:, k * CB2:(k + 1) * CB2]], axis=1))
        in_maps.append(im)

    nc = _build_program()
    res = run_bass_kernel_spmd(nc, in_maps, core_ids=list(range(N_CORES)),
                               trace=_trace)
    outs = res.results

    def gather(name, C):
        return np.concatenate(
            [_unpack(outs[k][f"po_{name}"], C) for k in range(N_CORES)], axis=1)

    new_w0 = gather("w0", CW0).reshape(NP_POP, HID, IN_D)
    new_w1 = gather("w1", CW1).reshape(NP_POP, HID, HID)
    new_w2 = gather("w2", CW2).reshape(NP_POP, OUT_D, HID)
    new_b0 = np.concatenate([outs[k]["o_b"][:, 0:CB0] for k in range(N_CORES)], axis=1)
    new_b1 = np.concatenate([outs[k]["o_b"][:, CB0:2 * CB0] for k in range(N_CORES)], axis=1)
    new_b2 = np.concatenate([outs[k]["o_b"][:, 2 * CB0:CB] for k in range(N_CORES)], axis=1)

    new_fx = np.where(accept, fy, fx).astype(f32)
    min_f = np.float32(new_fx.min())
    best_out = np.int32(np.argmin(new_fx))

    out = (new_w0, new_b0, new_w1, new_b1, new_w2, new_b2, new_fx, min_f, best_out)
    if _trace:
        return out, res
    return out


# revision 23
# speedup vs baseline: 2.2736x; 1.0215x over previous
"""DE/NN population-update kernel for Trainium2 (8 NeuronCores).

Reference computation (per parameter tensor p with uniform tensor ri):
    mutant = p + F*(p[best] - p) + F*(p[a0] - p[a1])        (gathers along NP axis)
    cond   = (ri < CR) | (Rs == layer)[:, None...]
    y      = where(cond, mutant, p)
    out    = where((fy <= fx)[:, None...], y, p)

Key transforms:
  * mutant = M @ p along the NP=44 axis with
        M[i,j] = (1-F)*d_ij + F*d[j==best] + F*d[j==a0[i]] - F*d[j==a1[i]]
    so the row gathers become one 88x88 block-diag matmul (2 column-blocks
    stacked) on the otherwise idle TensorEngine -- no extra HBM traffic.
  * cond & accept fold into a per-row threshold on ri:
        thr_i = -1.0 (reject row) / 0.9 (CR test) / 1.9 (forced crossover)
    so per element:  mask = (ri < thr_row); out = mask ? mutant : p
  * Sharding: every [NP, ...] tensor is split along flattened trailing dims
    across 8 cores; fx/fy/min/argmin are 44-element host-side ops.
  * DMA layout: HW-measured on trn2, only flat 2-D 128-partition DMAs reach
    HBM roofline (~376 GB/s vs ~170-210 for 44/88-partition or 3-D-AP
    transfers). So the host prepacks each core's slab into [128, nst*W]:
    super-tile s holds column-blocks (2s, 2s+1) on partition rows 0-43 /
    44-87, rows 88-127 are pad. 45% extra bytes, ~2x effective bandwidth.
"""

import numpy as np

import concourse.bacc as bacc
import concourse.mybir as mybir
from concourse.bass_utils import run_bass_kernel_spmd
from concourse.tile import TileContext

F_CONST = 0.8
CR_CONST = 0.9
NP_POP = 44
IN_D, HID, OUT_D = 512, 1024, 512
N_CORES = 8

CW0 = HID * IN_D // N_CORES     # 65536 cols/core
CW1 = HID * HID // N_CORES      # 131072
CW2 = OUT_D * HID // N_CORES    # 65536
CB0 = HID // N_CORES            # 128
CB2 = OUT_D // N_CORES          # 64
CB = 2 * CB0 + CB2              # 320 (b0|b1|b2 concat)

W = 2048                        # columns per block; super-tile = 2 blocks
MMN = 512                       # fp32 matmul moving-operand max (1 PSUM bank)
P2 = 2 * NP_POP                 # 88 live partitions
PFULL = 128

# (name, cols-per-core, layer index) for the big weight tensors
W_SPECS = (("w0", CW0, 0), ("w1", CW1, 1), ("w2", CW2, 2))

_PROGRAM = None


def _nst(C):
    return C // (2 * W)


def _build_program():
    """Trace the per-core Bass/Tile program (shapes only; values are inputs)."""
    global _PROGRAM
    if _PROGRAM is not None:
        return _PROGRAM

    nc = bacc.Bacc()
    f32 = mybir.dt.float32
    u8 = mybir.dt.uint8

    px, pr, po = {}, {}, {}
    for name, C, _ in W_SPECS:
        n = _nst(C)
        px[name] = nc.dram_tensor(f"px_{name}", [PFULL, n * W], f32, kind="ExternalInput")
        # ri pieces q=0,1 (32 rows each) ride densely in pr; the 24-row q=2
        # piece rides free in the x-tile pad slot (rows 96-119)
        pr[name] = nc.dram_tensor(f"pr_{name}", [PFULL, (n // 2) * W], f32,
                                  kind="ExternalInput")
        po[name] = nc.dram_tensor(f"po_{name}", [PFULL, n * W], f32, kind="ExternalOutput")
    xb = nc.dram_tensor("x_b", [NP_POP, CB], f32, kind="ExternalInput")
    rb = nc.dram_tensor("r_b", [NP_POP, CB], f32, kind="ExternalInput")
    ob = nc.dram_tensor("o_b", [NP_POP, CB], f32, kind="ExternalOutput")
    mt = nc.dram_tensor("mt", [P2, P2], f32, kind="ExternalInput")   # block-diag(M^T, M^T)
    th = nc.dram_tensor("th", [P2, 4], f32, kind="ExternalInput")    # col L = [thr_L; thr_L]

    with TileContext(nc) as tc:
        with (
            tc.tile_pool(name="const", bufs=1) as cpool,
            tc.tile_pool(name="xp", bufs=10) as xpool,
            tc.tile_pool(name="rp", bufs=9) as rpool,
            tc.tile_pool(name="mp", bufs=9) as mpool,
            tc.tile_pool(name="pp", bufs=8, space="PSUM") as ppool,
        ):
            mt_t = cpool.tile([P2, P2], f32, name="mt_t")
            nc.sync.dma_start(mt_t[:, :], mt[:, :])
            th_t = cpool.tile([P2, 4], f32, name="th_t")
            nc.sync.dma_start(th_t[:, :], th[:, :])

            # biases: one [44, 320] tile, per-layer column ranges 128|128|64
            xbt = xpool.tile([NP_POP, CB], f32, name="xbt", tag="xt")
            nc.gpsimd.dma_start(xbt[:, :], xb[:, :])
            rbt = rpool.tile([NP_POP, CB], f32, name="rbt", tag="rt")
            nc.gpsimd.dma_start(rbt[:, :], rb[:, :])
            psb = ppool.tile([NP_POP, CB], f32, name="psb", tag="ps")
            nc.tensor.matmul(
                psb[:, :], mt_t[0:NP_POP, 0:NP_POP], xbt[:, :],
                start=True, stop=True,
            )
            mkb = mpool.tile([NP_POP, CB], u8, name="mkb", tag="mk")
            bounds = (0, CB0, 2 * CB0, CB)
            for L in range(3):
                lo, hi = bounds[L], bounds[L + 1]
                nc.vector.tensor_scalar(
                    mkb[:, lo:hi], rbt[:, lo:hi], th_t[0:NP_POP, L:L + 1], None,
                    mybir.AluOpType.is_lt,
                )
            nc.vector.copy_predicated(xbt[:, :], mkb[:, :], psb[:, :])
            nc.gpsimd.dma_start(ob[:, :], xbt[:, :])


            for name, C, L in W_SPECS:
                rslices = {}

                def rslice(b, name=name):
                    if b not in rslices:
                        rt = rpool.tile([PFULL, W], f32, name="rt", tag="rt")
                        nc.gpsimd.dma_start(rt[:, :], pr[name][:, b * W:(b + 1) * W])
                        rslices[b] = rt
                    return rslices[b]

                for s in range(_nst(C)):
                    sl0 = slice(s * W, (s + 1) * W)
                    xt = xpool.tile([PFULL, W], f32, name="xt", tag="xt")
                    nc.gpsimd.dma_start(xt[:, :], px[name][:, sl0])
                    mk = mpool.tile([P2, W], u8, name="mk", tag="mk")
                    for q in (0, 1):
                        t = 2 * s + q
                        rt = rslice(t // 4)
                        ro = 32 * (t % 4)
                        mo = 32 * q
                        nc.vector.tensor_scalar(
                            mk[mo:mo + 32, :], rt[ro:ro + 32, :],
                            th_t[mo:mo + 32, L:L + 1], None,
                            mybir.AluOpType.is_lt,
                        )
                    nc.vector.tensor_scalar(
                        mk[64:88, :], xt[96:120, :],
                        th_t[64:88, L:L + 1], None,
                        mybir.AluOpType.is_lt,
                    )
                    for g in range(W // MMN):
                        sl = slice(g * MMN, (g + 1) * MMN)
                        ps = ppool.tile([P2, MMN], f32, name="ps", tag="ps")
                        nc.tensor.matmul(
                            ps[:, :], mt_t[:, :], xt[0:P2, sl],
                            start=True, stop=True,
                        )
                        nc.vector.copy_predicated(xt[0:P2, sl], mk[:, sl], ps[:, :])
                    nc.gpsimd.dma_start(po[name][:, sl0], xt[:, :])

    if not nc.is_finalized():
        nc.finalize()
    _PROGRAM = nc
    return nc


def _pack(slab, ri_slab):
    """[44, C] core slab -> [128, nst*W] super-tile layout: rows 0-87 = two
    column-blocks of x, rows 96-119 = ri pair-rows 64-87 (the q=2 piece)."""
    C = slab.shape[1]
    n = _nst(C)
    v = slab.reshape(NP_POP, n, 2, W)
    rv = ri_slab.reshape(NP_POP, n, 2, W)
    pack = np.zeros((PFULL, n, W), np.float32)
    pack[0:NP_POP] = v[:, :, 0, :]
    pack[NP_POP:P2] = v[:, :, 1, :]
    # ri pair-rows 64-87 = ri rows 20-43 of the second column-block
    pack[96:120] = rv[20:44, :, 1, :]
    return pack.reshape(PFULL, n * W)


def _pack_ri(slab):
    """[44, C] ri slab -> dense [128, (n/2)*W]: q=0,1 pieces in 32-row slots."""
    C = slab.shape[1]
    n = _nst(C)
    v = slab.reshape(NP_POP, n, 2, W)
    prow = np.empty((P2, n, W), np.float32)
    prow[0:NP_POP] = v[:, :, 0, :]
    prow[NP_POP:P2] = v[:, :, 1, :]
    # pieces [2n, 32, W]: piece 2s+q = pair-rows [32q, 32q+32) of tile s
    P = np.empty((2 * n, 32, W), np.float32)
    P[0::2] = prow[0:32].transpose(1, 0, 2)
    P[1::2] = prow[32:64].transpose(1, 0, 2)
    # slot t -> (partition 32*(t%4), column-block t//4)
    return np.ascontiguousarray(
        P.reshape(n // 2, 4, 32, W).transpose(1, 2, 0, 3)
    ).reshape(PFULL, (n // 2) * W)


def _unpack(packed, C):
    """[128, nst*W] -> [44, C]."""
    n = _nst(C)
    o3 = packed.reshape(PFULL, n, W)
    out = np.empty((NP_POP, C), np.float32)
    v = out.reshape(NP_POP, n, 2, W)
    v[:, :, 0, :] = o3[0:NP_POP]
    v[:, :, 1, :] = o3[NP_POP:P2]
    return out


def _host_side(fx, fy, Rs, a0, a1, best):
    """M^T (stacked block-diag) and the per-layer row thresholds."""
    f32 = np.float32
    idx = np.arange(NP_POP)
    M = np.zeros((NP_POP, NP_POP), np.float64)
    M[idx, idx] += 1.0 - F_CONST
    M[:, best] += F_CONST
    np.add.at(M, (idx, a0), F_CONST)
    np.add.at(M, (idx, a1), -F_CONST)
    MT = np.ascontiguousarray(M.T).astype(f32)
    mt_in = np.zeros((P2, P2), f32)
    mt_in[:NP_POP, :NP_POP] = MT
    mt_in[NP_POP:, NP_POP:] = MT

    accept = fy <= fx
    th_in = np.zeros((P2, 4), f32)
    for L in range(3):
        thr = np.where(accept, np.where(Rs == L, f32(1.9), f32(CR_CONST)), f32(-1.0))
        th_in[:NP_POP, L] = thr
        th_in[NP_POP:, L] = thr
    return mt_in, th_in, accept


def kernel(w0, b0, w1, b1, w2, b2,
           ri_w0, ri_b0, ri_w1, ri_b1, ri_w2, ri_b2,
           fx, fy, best_model, a0, a1, Rs,
           _trace=False):
    f32 = np.float32
    w0 = np.asarray(w0, f32); b0 = np.asarray(b0, f32)
    w1 = np.asarray(w1, f32); b1 = np.asarray(b1, f32)
    w2 = np.asarray(w2, f32); b2 = np.asarray(b2, f32)
    ri_w0 = np.asarray(ri_w0, f32); ri_b0 = np.asarray(ri_b0, f32)
    ri_w1 = np.asarray(ri_w1, f32); ri_b1 = np.asarray(ri_b1, f32)
    ri_w2 = np.asarray(ri_w2, f32); ri_b2 = np.asarray(ri_b2, f32)
    fx = np.asarray(fx, f32); fy = np.asarray(fy, f32)
    a0 = np.asarray(a0, np.int64); a1 = np.asarray(a1, np.int64)
    Rs = np.asarray(Rs, np.int64)
    best = int(best_model)

    mt_in, th_in, accept = _host_side(fx, fy, Rs, a0, a1, best)

    wf = {"w0": w0.reshape(NP_POP, -1), "w1": w1.reshape(NP_POP, -1),
          "w2": w2.reshape(NP_POP, -1)}
    rf = {"w0": ri_w0.reshape(NP_POP, -1), "w1": ri_w1.reshape(NP_POP, -1),
          "w2": ri_w2.reshape(NP_POP, -1)}

    in_maps = []
    for k in range(N_CORES):
        im = {"mt": mt_in, "th": th_in}
        for name, C, _ in W_SPECS:
            im[f"px_{name}"] = _pack(wf[name][:, k * C:(k + 1) * C],
                                     rf[name][:, k * C:(k + 1) * C])
            im[f"pr_{name}"] = _pack_ri(rf[name][:, k * C:(k + 1) * C])
        im["x_b"] = np.ascontiguousarray(np.concatenate(
            [b0[:, k * CB0:(k + 1) * CB0], b1[:, k * CB0:(k + 1) * CB0],
             b2[:, k * CB2:(k + 1) * CB2]], axis=1))
        im["r_b"] = np.ascontiguousarray(np.concatenate(
            [ri_b0[:, k * CB0:(k + 1) * CB0], ri_b1[:, k * CB0:(k + 1) * CB0],
             ri_b2[:, k * CB2:(k + 1) * CB2]], axis=1))
        in_maps.append(im)

    nc = _build_program()
    res = run_bass_kernel_spmd(nc, in_maps, core_ids=list(range(N_CORES)),
                               trace=_trace)
    outs = res.results

    def gather(name, C):
        return np.concatenate(
            [_unpack(outs[k][f"po_{name}"], C) for k in range(N_CORES)], axis=1)

    new_w0 = gather("w0", CW0).reshape(NP_POP, HID, IN_D)
    new_w1 = gather("w1", CW1).reshape(NP_POP, HID, HID)
    new_w2 = gather("w2", CW2).reshape(NP_POP, OUT_D, HID)
    new_b0 = np.concatenate([outs[k]["o_b"][:, 0:CB0] for k in range(N_CORES)], axis=1)
    new_b1 = np.concatenate([outs[k]["o_b"][:, CB0:2 * CB0] for k in range(N_CORES)], axis=1)
    new_b2 = np.concatenate([outs[k]["o_b"][:, 2 * CB0:CB] for k in range(N_CORES)], axis=1)

    new_fx = np.where(accept, fy, fx).astype(f32)
    min_f = np.float32(new_fx.min())
    best_out = np.int32(np.argmin(new_fx))

    out = (new_w0, new_b0, new_w1, new_b1, new_w2, new_b2, new_fx, min_f, best_out)
    if _trace:
        return out, res
    return out
